# revision 8
# baseline (speedup 1.0000x reference)
"""Chamfer L2 distance kernel for 8 Trainium2 NeuronCores.

Strategy (data-parallel over batch, 2 batches/core), single-matrix design:
  For each batch the device computes the full squared-distance matrix
  D[n, m] = |x_n|^2 - 2<x_n, y_m> + |y_m|^2 ONCE via K=24 bf16 matmuls
  (exact hi/mid/lo bf16 decomposition, ~1e-7 absolute error), then extracts
  BOTH chamfer reductions from the same PSUM data with a fused custom DVE op:
    body out  = min(Src0, Src1)          -> running column-min tile (SBUF)
    accum_out = min over free of Src0    -> per-partition row-min slot
  so every element is read from PSUM exactly once.  A fraction of the
  n-tiles is offloaded through the Scalar engine (PSUM fp32 -> SBUF fp16
  copy) and consumed by a hand-authored 2x_1PORT uop program of the same op
  (2 elem/cycle/lane), which roughly balances Scalar and Vector engine time.
  Host finishes: row-min means + column partial-min -> min over partitions,
  means in fp64.

Self-contained: hardcodes B=16, N=M=4096, C=3, 8 cores.
"""

import numpy as np
import ml_dtypes

BF = ml_dtypes.bfloat16
B, N, M, C = 16, 4096, 4096, 3
NCORES = 8
BPC = B // NCORES          # batches per core
K = 24                     # contraction rows (18 product + 3 ynorm + 3 xnorm)
NT = N // 128              # n-tiles per batch
HW_TILE = 2048             # free width of one PSUM tile (2 per n-tile)
NH = M // HW_TILE          # m-halves per n-tile (2)
# n-tile indices consumed via the Scalar-copy fp16 2x path (rest read PSUM
# directly at 1x).  Tuned so ScalarE copy time ~ VectorE reduce time.
OFFLOAD_MOD, OFFLOAD_KEEP = 8, (0, 3, 6)   # i%8 in KEEP -> direct, else offload
import os as _os
if _os.environ.get("CHAMFER_KEEP"):        # tuning knob, e.g. "0,2,4,6"
    OFFLOAD_KEEP = tuple(int(x) for x in _os.environ["CHAMFER_KEEP"].split(","))


def _is_offload(i):
    return (i % OFFLOAD_MOD) not in OFFLOAD_KEEP


_CACHE = {}


# ---------------------------------------------------------------- host prep --

def _split3(v):
    """Exact-ish 3-way bf16 decomposition: h + m + l = v + O(2^-27 |v|)."""
    h = v.astype(BF)
    r = v - h.astype(np.float64)
    m = r.astype(BF)
    r2 = r - m.astype(np.float64)
    l = r2.astype(BF)
    return h, m, l


def _build_tabs(X, Y):
    """X: (N,3) partition side, Y: (M,3) free side.
    Returns lhsT (24, N) bf16 and rhs (24, M) bf16 such that
    (lhsT.T @ rhs)[n, m] ~= |X_n|^2 - 2<X_n, Y_m> + |Y_m|^2 to ~1e-7 abs."""
    lt = np.empty((K, X.shape[0]), BF)
    rt = np.empty((K, Y.shape[0]), BF)
    Xd = X.astype(np.float64)
    Yd = -2.0 * Y.astype(np.float64)
    row = 0
    for c in range(C):
        Xh, Xm, Xl = _split3(Xd[:, c])
        Yh, Ym, Yl = _split3(Yd[:, c])
        for a, b in ((Xh, Yh), (Xh, Ym), (Xm, Yh), (Xm, Ym), (Xh, Yl), (Xl, Yh)):
            lt[row] = a
            rt[row] = b
            row += 1
    ones_n = np.ones(X.shape[0], BF)
    ones_m = np.ones(Y.shape[0], BF)
    q = np.sum(Y.astype(np.float64) ** 2, axis=1)
    for qq in _split3(q):
        lt[row] = ones_n
        rt[row] = qq
        row += 1
    p = np.sum(X.astype(np.float64) ** 2, axis=1)
    for pp in _split3(p):
        lt[row] = pp
        rt[row] = ones_m
        row += 1
    assert row == K
    return lt, rt


# ------------------------------------------------------- fused custom DVE op --

def _get_fused_op():
    """CHAMFER_FUSED_ANT: out = min(in0, in1); accum_out = min_free(in0) ^ s0.

    Hand-built uop programs (the Spec DSL can only reduce the body output,
    we need the accumulator to reduce Src0 alone):
      REGULAR (1x):  any dtype, PSUM in0 supported.
      2X_1PORT:      16-bit dtypes, 2 elems/cycle/lane.
    """
    if "op" in _CACHE:
        return _CACHE["op"]
    import concourse.dve_ops as dve_ops_mod
    from concourse.dve_ops import DveOp
    from concourse.dve_spec import Spec, Src0, Src1, C0, minn
    from concourse.dve_uop import (
        DveOpSpec, UopConfig, AluOp, AluInp, InpSel, OutSel, OutPath,
        Trigger, DelayInp, ENABLE,
    )

    name = "CHAMFER_FUSED_ANT"
    for op in dve_ops_mod.OPS:
        if op.name == name:
            _CACHE["op"] = op
            return op

    def _ref(in0, in1, s0, s1, imm2):
        a = np.asarray(in0, np.float32)
        b = np.asarray(in1, np.float32)
        out = np.minimum(a, b)
        s0a = (np.asarray(s0, np.float32).reshape(-1, 1)
               if isinstance(s0, np.ndarray) else np.float32(s0))
        acc = np.minimum(a.reshape(a.shape[0], -1).min(axis=-1, keepdims=True), s0a)
        return out, acc

    spec = Spec(body=minn(Src0, Src1), accum=minn, accum_init=C0, reference=_ref)

    if name not in dve_ops_mod._SUB_OPCODE_FOR_NAME:
        row = max(dve_ops_mod._SUB_OPCODE_FOR_NAME.values()) + 1
        assert row < 0x20
        dve_ops_mod._SUB_OPCODE_FOR_NAME[name] = row
    opcode = dve_ops_mod.get_dve_sub_opcode(name)

    SRC_DONE = (Trigger.SRC_TENSOR_DONE, Trigger.NONE, Trigger.NONE)
    COUNT_ONCE = (Trigger.COUNT, Trigger.NONE, Trigger.NONE)
    PD = [AluInp.PREV_DELAY_0, AluInp.PREV_DELAY_1, AluInp.PREV_DELAY_2,
          AluInp.PREV_DELAY_3, AluInp.PREV_DELAY_4, AluInp.PREV_DELAY_5]
    PREV = AluInp.PREV_ALU_OUT
    CURR = AluInp.CURR_ALU_OUT
    DELAY_OUT = [OutSel.DELAY_0, OutSel.DELAY_1, OutSel.DELAY_2,
                 OutSel.DELAY_3, OutSel.DELAY_4, OutSel.DELAY_5]

    def mk_uop(inp_map, req, trigger, nxt, repeat):
        u = UopConfig()
        for port, sel in inp_map.items():
            u.enable_input(sel, port)
        u.require_inp0, u.require_inp1 = req
        u.trigger = trigger
        u.next_uop = nxt
        u.repeat_count = repeat
        u.accum_enabled = ENABLE
        return u

    # ---- REGULAR program: lanes 0=Src0, 1=Src1, 2=C0; root captured ->lane0
    inp1x = {1: InpSel.SRC_0, 2: InpSel.SRC_1, 3: InpSel.CONST_0}
    lanes1x = (0, 1, 2)

    steady1 = mk_uop(inp1x, (1, 1), SRC_DONE, (0, 0, 0), 0)
    dp = steady1.datapath_config
    dp[0].enable_alu(AluOp.MIN, PD[0], PD[1]).pass_through_delay(*lanes1x)
    # accum reduces Src0 (lane0 read happens before the same-stage capture)
    dp[1].enable_alu(AluOp.MIN, CURR, PD[0]).pass_through_delay(1, 2)
    dp[1].enable_delay_from_src(DelayInp.PREV_ALU_OUT, 0)
    dp[1].alu_out_a_enable = ENABLE
    for k in range(2, 8):
        dp[k].enable_alu(AluOp.BYPASS, PREV).pass_through_delay(*lanes1x)
        dp[k].alu_out_a_enable = ENABLE
    steady1.enable_output(DELAY_OUT[0], OutPath.WR0_LO)

    seed1 = mk_uop(inp1x, (0, 0), COUNT_ONCE, (1, 0, 0), 1)
    dp = seed1.datapath_config
    dp[0].enable_alu(AluOp.MIN, PD[0], PD[1]).pass_through_delay(*lanes1x)
    dp[1].enable_alu(AluOp.BYPASS, PD[2]).pass_through_delay(1, 2)
    dp[1].enable_delay_from_src(DelayInp.PREV_ALU_OUT, 0)
    dp[1].alu_out_a_enable = ENABLE
    for k in range(2, 8):
        dp[k].enable_alu(AluOp.BYPASS, PREV).pass_through_delay(*lanes1x)
        dp[k].alu_out_a_enable = ENABLE

    # ---- 2X_1PORT program: lanes 0=Src0 1=Src1 2=C0 3=Src0Hi 4=Src1Hi
    #      r_lo = min(S0,S1) captured ->lane1@blk1; r_hi ->lane3@blk2
    #      accum = min(acc, min(S0, S0Hi)) at blk3
    inp2x = {1: InpSel.SRC_0, 2: InpSel.SRC_1, 3: InpSel.CONST_0,
             4: InpSel.SRC_0_HI, 5: InpSel.SRC_1_HI}
    lanes2x = (0, 1, 2, 3, 4)

    steady2 = mk_uop(inp2x, (1, 1), SRC_DONE, (0, 0, 0), 0)
    dp = steady2.datapath_config
    dp[0].enable_alu(AluOp.MIN, PD[0], PD[1]).pass_through_delay(*lanes2x)
    dp[1].enable_alu(AluOp.MIN, PD[3], PD[4]).pass_through_delay(0, 2, 3, 4)
    dp[1].enable_delay_from_src(DelayInp.PREV_ALU_OUT, 1)          # r_lo
    dp[2].enable_alu(AluOp.MIN, PD[0], PD[3]).pass_through_delay(0, 1, 2, 4)
    dp[2].enable_delay_from_src(DelayInp.PREV_ALU_OUT, 3)          # r_hi
    dp[3].enable_alu(AluOp.MIN, CURR, PREV).pass_through_delay(*lanes2x)
    dp[3].alu_out_a_enable = ENABLE
    for k in range(4, 8):
        dp[k].enable_alu(AluOp.BYPASS, PREV).pass_through_delay(*lanes2x)
        dp[k].alu_out_a_enable = ENABLE
    steady2.enable_output(DELAY_OUT[1], OutPath.WR0_LO)
    steady2.enable_output(DELAY_OUT[3], OutPath.WR0_HI)

    seed2 = mk_uop(inp2x, (0, 0), COUNT_ONCE, (1, 0, 0), 1)
    dp = seed2.datapath_config
    for k in range(3):
        dp[k].enable_alu(AluOp.BYPASS, PREV).pass_through_delay(*lanes2x)
    dp[3].enable_alu(AluOp.BYPASS, PD[2]).pass_through_delay(*lanes2x)
    dp[3].alu_out_a_enable = ENABLE
    for k in range(4, 8):
        dp[k].enable_alu(AluOp.BYPASS, PREV).pass_through_delay(*lanes2x)
        dp[k].alu_out_a_enable = ENABLE

    compiled = DveOpSpec(
        name=name,
        opcode=opcode,
        uops=[seed1, steady1],
        uops_2x=[seed2, steady2],
        perf_max=1,
        rd1_en=True,
    )
    compiled.validate("v3")
    dve_ops_mod._COMPILE_CACHE[(name, "v3")] = compiled

    op = DveOp(name, spec, False, {"v3": compiled.sha("v3")})
    dve_ops_mod.OPS.append(op)
    dve_ops_mod.CUSTOM_DVE_SPECS[name] = spec
    _CACHE["op"] = op
    return op


def _emit_fused(nc, op, *, out, in0, in1, s0, accum_out):
    """nc.vector._custom_dve clone that sets perf_max=1 on the instruction
    (byte-36[7:6]) so the engine may engage the 2X_1PORT program."""
    import concourse.bass_isa as bass_isa
    import concourse.mybir as mybir
    from concourse.dve_ops import get_dve_sub_opcode

    v = nc.vector
    if op.name not in v.bass.m.ant_custom_dve_ops:
        v.bass.m.ant_custom_dve_ops = sorted(
            {*v.bass.m.ant_custom_dve_ops, op.name}
        )
    shape = bass_isa.CustomDveShape.TTSS
    isa_opcode = v.bass.isa.Opcode[
        f"NEURON_ISA_TPB_OPCODE_CUSTOM_DVE_ANT_{shape.slot()}"
    ].value
    imm = lambda x: mybir.ImmediateValue(dtype=mybir.dt.float32, value=float(x))
    ins = [
        v.lower_ap(in0, for_isa=True, opt=True),
        v.lower_ap(in1, for_isa=True, opt=True),
        imm(s0),
        imm(0.0),
    ]
    outs = [v.lower_ap(out, for_isa=True, opt=True),
            v.lower_ap(accum_out, for_isa=True)]
    return v.add_instruction(
        bass_isa.InstCustomDveAnt(
            name=v.bass.get_next_instruction_name(),
            op_name=op.name,
            rd1_en=True,
            subdim=0,
            imm2=0.0,
            shape=shape,
            row=get_dve_sub_opcode(op.name),
            isa_opcode=isa_opcode,
            ins=ins,
            outs=outs,
            perf_max=1,
        )
    )


# ------------------------------------------------------------- device build --

def _build_nc(reps=1):
    key = ("nc", reps)
    if key in _CACHE:
        return _CACHE[key]
    import concourse.bacc as bacc
    import concourse.mybir as mybir
    from concourse.tile import TileContext

    FUSED = _get_fused_op()
    f32 = mybir.dt.float32
    f16 = mybir.dt.float16
    bf16 = mybir.dt.bfloat16

    RAW_SLOTS = BPC * NT * NH            # row-min slots
    CM_SLOTS = BPC * NH                  # column-min output tiles

    nc = bacc.Bacc(None)
    ltab = nc.dram_tensor("ltab", [BPC, K, N], bf16, kind="ExternalInput")
    rtab = nc.dram_tensor("rtab", [BPC, K, M], bf16, kind="ExternalInput")
    # 2x-mode accum writeback is 16-bit; direct 1x tiles use fp32 slots.
    outr = nc.dram_tensor("raw", [128, RAW_SLOTS], f32, kind="ExternalOutput")
    outr16 = nc.dram_tensor("raw16", [128, RAW_SLOTS], f16,
                            kind="ExternalOutput")
    outc = nc.dram_tensor("colm", [128, CM_SLOTS * HW_TILE], f16,
                          kind="ExternalOutput")

    with TileContext(nc) as tc:
        with (
            tc.tile_pool(name="stage", bufs=2) as stage,
            tc.tile_pool(name="psum", bufs=2, space="PSUM") as psum,
            tc.tile_pool(name="fp16", bufs=4) as fpool,
            tc.tile_pool(name="res", bufs=1) as res,
        ):
            raw = res.tile([128, RAW_SLOTS], f32, tag="raw")
            raw16 = res.tile([128, RAW_SLOTS], f16, tag="raw16")
            nc.vector.memset(raw[:, :], 3.0e38)
            nc.vector.memset(raw16[:, :], 60000.0)
            cms = [res.tile([128, HW_TILE], f16, tag=f"cm{s}_{pp}",
                            name=f"cm{s}_{pp}")
                   for s in range(CM_SLOTS) for pp in range(2)]
            for t in cms:
                nc.vector.memset(t[:, :], 60000.0)

            for _rep in range(reps):
              for lb in range(BPC):
                lt = stage.tile([K, N], bf16, tag="lt")
                rt = stage.tile([K, M], bf16, tag="rt")
                nc.sync.dma_start(out=lt[:, :], in_=ltab[lb])
                nc.sync.dma_start(out=rt[:, :], in_=rtab[lb])
                for i in range(NT):
                    ltT = lt[:, i * 128:(i + 1) * 128]
                    for h in range(NH):
                        pa = psum.tile([128, HW_TILE], f32, tag="pa")
                        base = h * HW_TILE
                        for j in range(HW_TILE // 512):
                            nc.tensor.matmul(
                                pa[:, j * 512:(j + 1) * 512], ltT,
                                rt[:, base + j * 512:base + (j + 1) * 512],
                                start=True, stop=True)
                        slot = (lb * NT + i) * NH + h
                        cslot = (lb * NH + h) * 2
                        c_in = cms[cslot + (i % 2)]
                        c_out = cms[cslot + ((i + 1) % 2)]
                        if _is_offload(i):
                            fb = fpool.tile([128, HW_TILE], f16, tag="fb")
                            nc.scalar.copy(out=fb[:, :], in_=pa[:, :])
                            src, racc = fb, raw16
                        else:
                            src, racc = pa, raw
                        _emit_fused(
                            nc, FUSED,
                            out=c_out[:, :],
                            in0=src[:, :],
                            in1=c_in[:, :],
                            s0=3.0e38,
                            accum_out=racc[:, slot:slot + 1],
                        )
            # final column-min partials live in parity NT%2 (== 0)
            for s in range(CM_SLOTS):
                fin = cms[s * 2 + (NT % 2)]
                nc.sync.dma_start(
                    out=outc[:, s * HW_TILE:(s + 1) * HW_TILE], in_=fin[:, :])
            nc.sync.dma_start(out=outr[:, :], in_=raw[:, :])
            nc.sync.dma_start(out=outr16[:, :], in_=raw16[:, :])
    nc.compile()
    _CACHE[key] = nc
    return nc


# -------------------------------------------------------------------- entry --

def _prepare_inputs(pred, target):
    ltabs = np.empty((NCORES, BPC, K, N), BF)
    rtabs = np.empty((NCORES, BPC, K, M), BF)
    for core in range(NCORES):
        for lb in range(BPC):
            b = core * BPC + lb
            lt, rt = _build_tabs(pred[b], target[b])
            ltabs[core, lb] = lt
            rtabs[core, lb] = rt
    return ltabs, rtabs


def _postprocess(results):
    losses = []
    for core in range(NCORES):
        raw = np.minimum(
            np.asarray(results[core]["raw"], np.float64),
            np.asarray(results[core]["raw16"], np.float64))   # (128, RAW_SLOTS)
        colm = np.asarray(results[core]["colm"], np.float64)  # (128, CM*2048)
        for lb in range(BPC):
            sl = raw[:, lb * NT * NH:(lb + 1) * NT * NH]
            rowmin = sl.reshape(128, NT, NH).min(axis=2)      # (p, i)
            rowmin = rowmin.T.reshape(-1)                     # n = i*128 + p
            cm = colm[:, lb * NH * HW_TILE:(lb + 1) * NH * HW_TILE]
            colmin = cm.min(axis=0)                           # (M,)
            losses.append(rowmin.mean() + colmin.mean())
    return np.float32(np.mean(losses))


def _run(pred, target, trace=False):
    from concourse.bass_utils import run_bass_kernel_spmd

    pred = np.asarray(pred, dtype=np.float32)
    target = np.asarray(target, dtype=np.float32)
    assert pred.shape == (B, N, C) and target.shape == (B, M, C)
    ltabs, rtabs = _prepare_inputs(pred, target)
    nc = _build_nc()
    in_maps = [{"ltab": ltabs[c], "rtab": rtabs[c]} for c in range(NCORES)]
    res = run_bass_kernel_spmd(nc, in_maps, core_ids=list(range(NCORES)), trace=trace)
    return _postprocess(res.results), res


def kernel(pred, target):
    loss, _ = _run(pred, target, trace=False)
    return loss


# revision 13
# speedup vs baseline: 1.0768x; 1.0768x over previous
"""Chamfer L2 distance kernel for 8 Trainium2 NeuronCores.

Strategy (data-parallel over batch, 2 batches/core), single-matrix design:
  For each batch the device computes the full squared-distance matrix
  D[n, m] = |x_n|^2 - 2<x_n, y_m> + |y_m|^2 ONCE via K=24 bf16 matmuls
  (exact hi/mid/lo bf16 decomposition, ~1e-7 absolute error), then extracts
  BOTH chamfer reductions from the same PSUM data with a fused custom DVE op:
    body out  = min(Src0, Src1)          -> running column-min tile (SBUF)
    accum_out = min over free of Src0    -> per-partition row-min slot
  so every element is read from PSUM exactly once.  A fraction of the
  n-tiles is offloaded through the Scalar engine (PSUM fp32 -> SBUF fp16
  copy) and consumed by a hand-authored 2x_1PORT uop program of the same op
  (2 elem/cycle/lane), which roughly balances Scalar and Vector engine time.
  Host finishes: row-min means + column partial-min -> min over partitions,
  means in fp64.

Self-contained: hardcodes B=16, N=M=4096, C=3, 8 cores.
"""

import numpy as np
import ml_dtypes

BF = ml_dtypes.bfloat16
B, N, M, C = 16, 4096, 4096, 3
NCORES = 8
BPC = B // NCORES          # batches per core
K = 24                     # contraction rows (18 product + 3 ynorm + 3 xnorm)
NT = N // 128              # n-tiles per batch
HW_TILE = 2048             # free width of one PSUM tile (2 per n-tile)
NH = M // HW_TILE          # m-halves per n-tile (2)
# n-tile indices consumed via the Scalar-copy fp16 2x path (rest read PSUM
# directly at 1x).  Tuned so ScalarE copy time ~ VectorE reduce time at the
# documented 0.96 GHz DVE / 1.2 GHz ScalarE clocks.  Offloaded n-tiles are
# processed as ONE [128, 4096] fused op (fewer DVE ops -> less drain).
OFFLOAD_MOD, OFFLOAD_KEEP = 4, (0,)        # i%4 in KEEP -> direct, else offload
import os as _os
if _os.environ.get("CHAMFER_KEEP"):        # tuning knob, e.g. "0,2,4,6"
    OFFLOAD_KEEP = tuple(int(x) for x in _os.environ["CHAMFER_KEEP"].split(","))


def _is_offload(i):
    return (i % OFFLOAD_MOD) not in OFFLOAD_KEEP


_CACHE = {}


# ---------------------------------------------------------------- host prep --

def _split3(v):
    """Exact-ish 3-way bf16 decomposition: h + m + l = v + O(2^-27 |v|)."""
    h = v.astype(BF)
    r = v - h.astype(np.float64)
    m = r.astype(BF)
    r2 = r - m.astype(np.float64)
    l = r2.astype(BF)
    return h, m, l


def _build_tabs(X, Y):
    """X: (N,3) partition side, Y: (M,3) free side.
    Returns lhsT (24, N) bf16 and rhs (24, M) bf16 such that
    (lhsT.T @ rhs)[n, m] ~= |X_n|^2 - 2<X_n, Y_m> + |Y_m|^2 to ~1e-7 abs."""
    lt = np.empty((K, X.shape[0]), BF)
    rt = np.empty((K, Y.shape[0]), BF)
    Xd = X.astype(np.float64)
    Yd = -2.0 * Y.astype(np.float64)
    row = 0
    for c in range(C):
        Xh, Xm, Xl = _split3(Xd[:, c])
        Yh, Ym, Yl = _split3(Yd[:, c])
        for a, b in ((Xh, Yh), (Xh, Ym), (Xm, Yh), (Xm, Ym), (Xh, Yl), (Xl, Yh)):
            lt[row] = a
            rt[row] = b
            row += 1
    ones_n = np.ones(X.shape[0], BF)
    ones_m = np.ones(Y.shape[0], BF)
    q = np.sum(Y.astype(np.float64) ** 2, axis=1)
    for qq in _split3(q):
        lt[row] = ones_n
        rt[row] = qq
        row += 1
    p = np.sum(X.astype(np.float64) ** 2, axis=1)
    for pp in _split3(p):
        lt[row] = pp
        rt[row] = ones_m
        row += 1
    assert row == K
    return lt, rt


# ------------------------------------------------------- fused custom DVE op --

def _get_fused_op():
    """CHAMFER_FUSED_ANT: out = min(in0, in1); accum_out = min_free(in0) ^ s0.

    Hand-built uop programs (the Spec DSL can only reduce the body output,
    we need the accumulator to reduce Src0 alone):
      REGULAR (1x):  any dtype, PSUM in0 supported.
      2X_1PORT:      16-bit dtypes, 2 elems/cycle/lane.
    """
    if "op" in _CACHE:
        return _CACHE["op"]
    import concourse.dve_ops as dve_ops_mod
    from concourse.dve_ops import DveOp
    from concourse.dve_spec import Spec, Src0, Src1, C0, minn
    from concourse.dve_uop import (
        DveOpSpec, UopConfig, AluOp, AluInp, InpSel, OutSel, OutPath,
        Trigger, DelayInp, ENABLE,
    )

    name = "CHAMFER_FUSED_ANT"
    for op in dve_ops_mod.OPS:
        if op.name == name:
            _CACHE["op"] = op
            return op

    def _ref(in0, in1, s0, s1, imm2):
        a = np.asarray(in0, np.float32)
        b = np.asarray(in1, np.float32)
        out = np.minimum(a, b)
        s0a = (np.asarray(s0, np.float32).reshape(-1, 1)
               if isinstance(s0, np.ndarray) else np.float32(s0))
        acc = np.minimum(a.reshape(a.shape[0], -1).min(axis=-1, keepdims=True), s0a)
        return out, acc

    spec = Spec(body=minn(Src0, Src1), accum=minn, accum_init=C0, reference=_ref)

    if name not in dve_ops_mod._SUB_OPCODE_FOR_NAME:
        row = max(dve_ops_mod._SUB_OPCODE_FOR_NAME.values()) + 1
        assert row < 0x20
        dve_ops_mod._SUB_OPCODE_FOR_NAME[name] = row
    opcode = dve_ops_mod.get_dve_sub_opcode(name)

    SRC_DONE = (Trigger.SRC_TENSOR_DONE, Trigger.NONE, Trigger.NONE)
    COUNT_ONCE = (Trigger.COUNT, Trigger.NONE, Trigger.NONE)
    PD = [AluInp.PREV_DELAY_0, AluInp.PREV_DELAY_1, AluInp.PREV_DELAY_2,
          AluInp.PREV_DELAY_3, AluInp.PREV_DELAY_4, AluInp.PREV_DELAY_5]
    PREV = AluInp.PREV_ALU_OUT
    CURR = AluInp.CURR_ALU_OUT
    DELAY_OUT = [OutSel.DELAY_0, OutSel.DELAY_1, OutSel.DELAY_2,
                 OutSel.DELAY_3, OutSel.DELAY_4, OutSel.DELAY_5]

    def mk_uop(inp_map, req, trigger, nxt, repeat):
        u = UopConfig()
        for port, sel in inp_map.items():
            u.enable_input(sel, port)
        u.require_inp0, u.require_inp1 = req
        u.trigger = trigger
        u.next_uop = nxt
        u.repeat_count = repeat
        u.accum_enabled = ENABLE
        return u

    # ---- REGULAR program: lanes 0=Src0, 1=Src1, 2=C0; root captured ->lane0
    inp1x = {1: InpSel.SRC_0, 2: InpSel.SRC_1, 3: InpSel.CONST_0}
    lanes1x = (0, 1, 2)

    steady1 = mk_uop(inp1x, (1, 1), SRC_DONE, (0, 0, 0), 0)
    dp = steady1.datapath_config
    dp[0].enable_alu(AluOp.MIN, PD[0], PD[1]).pass_through_delay(*lanes1x)
    # accum reduces Src0 (lane0 read happens before the same-stage capture)
    dp[1].enable_alu(AluOp.MIN, CURR, PD[0]).pass_through_delay(1, 2)
    dp[1].enable_delay_from_src(DelayInp.PREV_ALU_OUT, 0)
    dp[1].alu_out_a_enable = ENABLE
    for k in range(2, 8):
        dp[k].enable_alu(AluOp.BYPASS, PREV).pass_through_delay(*lanes1x)
        dp[k].alu_out_a_enable = ENABLE
    steady1.enable_output(DELAY_OUT[0], OutPath.WR0_LO)

    seed1 = mk_uop(inp1x, (0, 0), COUNT_ONCE, (1, 0, 0), 1)
    dp = seed1.datapath_config
    dp[0].enable_alu(AluOp.MIN, PD[0], PD[1]).pass_through_delay(*lanes1x)
    dp[1].enable_alu(AluOp.BYPASS, PD[2]).pass_through_delay(1, 2)
    dp[1].enable_delay_from_src(DelayInp.PREV_ALU_OUT, 0)
    dp[1].alu_out_a_enable = ENABLE
    for k in range(2, 8):
        dp[k].enable_alu(AluOp.BYPASS, PREV).pass_through_delay(*lanes1x)
        dp[k].alu_out_a_enable = ENABLE

    # ---- 2X_1PORT program: lanes 0=Src0 1=Src1 2=C0 3=Src0Hi 4=Src1Hi
    #      r_lo = min(S0,S1) captured ->lane1@blk1; r_hi ->lane3@blk2
    #      accum = min(acc, min(S0, S0Hi)) at blk3
    inp2x = {1: InpSel.SRC_0, 2: InpSel.SRC_1, 3: InpSel.CONST_0,
             4: InpSel.SRC_0_HI, 5: InpSel.SRC_1_HI}
    lanes2x = (0, 1, 2, 3, 4)

    steady2 = mk_uop(inp2x, (1, 1), SRC_DONE, (0, 0, 0), 0)
    dp = steady2.datapath_config
    dp[0].enable_alu(AluOp.MIN, PD[0], PD[1]).pass_through_delay(*lanes2x)
    dp[1].enable_alu(AluOp.MIN, PD[3], PD[4]).pass_through_delay(0, 2, 3, 4)
    dp[1].enable_delay_from_src(DelayInp.PREV_ALU_OUT, 1)          # r_lo
    dp[2].enable_alu(AluOp.MIN, PD[0], PD[3]).pass_through_delay(0, 1, 2, 4)
    dp[2].enable_delay_from_src(DelayInp.PREV_ALU_OUT, 3)          # r_hi
    dp[3].enable_alu(AluOp.MIN, CURR, PREV).pass_through_delay(*lanes2x)
    dp[3].alu_out_a_enable = ENABLE
    for k in range(4, 8):
        dp[k].enable_alu(AluOp.BYPASS, PREV).pass_through_delay(*lanes2x)
        dp[k].alu_out_a_enable = ENABLE
    steady2.enable_output(DELAY_OUT[1], OutPath.WR0_LO)
    steady2.enable_output(DELAY_OUT[3], OutPath.WR0_HI)

    seed2 = mk_uop(inp2x, (0, 0), COUNT_ONCE, (1, 0, 0), 1)
    dp = seed2.datapath_config
    for k in range(3):
        dp[k].enable_alu(AluOp.BYPASS, PREV).pass_through_delay(*lanes2x)
    dp[3].enable_alu(AluOp.BYPASS, PD[2]).pass_through_delay(*lanes2x)
    dp[3].alu_out_a_enable = ENABLE
    for k in range(4, 8):
        dp[k].enable_alu(AluOp.BYPASS, PREV).pass_through_delay(*lanes2x)
        dp[k].alu_out_a_enable = ENABLE

    compiled = DveOpSpec(
        name=name,
        opcode=opcode,
        uops=[seed1, steady1],
        uops_2x=[seed2, steady2],
        perf_max=1,
        rd1_en=True,
    )
    compiled.validate("v3")
    dve_ops_mod._COMPILE_CACHE[(name, "v3")] = compiled

    op = DveOp(name, spec, False, {"v3": compiled.sha("v3")})
    dve_ops_mod.OPS.append(op)
    dve_ops_mod.CUSTOM_DVE_SPECS[name] = spec
    _CACHE["op"] = op
    return op


def _emit_fused(nc, op, *, out, in0, in1, s0, accum_out):
    """nc.vector._custom_dve clone that sets perf_max=1 on the instruction
    (byte-36[7:6]) so the engine may engage the 2X_1PORT program."""
    import concourse.bass_isa as bass_isa
    import concourse.mybir as mybir
    from concourse.dve_ops import get_dve_sub_opcode

    v = nc.vector
    if op.name not in v.bass.m.ant_custom_dve_ops:
        v.bass.m.ant_custom_dve_ops = sorted(
            {*v.bass.m.ant_custom_dve_ops, op.name}
        )
    shape = bass_isa.CustomDveShape.TTSS
    isa_opcode = v.bass.isa.Opcode[
        f"NEURON_ISA_TPB_OPCODE_CUSTOM_DVE_ANT_{shape.slot()}"
    ].value
    imm = lambda x: mybir.ImmediateValue(dtype=mybir.dt.float32, value=float(x))
    ins = [
        v.lower_ap(in0, for_isa=True, opt=True),
        v.lower_ap(in1, for_isa=True, opt=True),
        imm(s0),
        imm(0.0),
    ]
    outs = [v.lower_ap(out, for_isa=True, opt=True),
            v.lower_ap(accum_out, for_isa=True)]
    return v.add_instruction(
        bass_isa.InstCustomDveAnt(
            name=v.bass.get_next_instruction_name(),
            op_name=op.name,
            rd1_en=True,
            subdim=0,
            imm2=0.0,
            shape=shape,
            row=get_dve_sub_opcode(op.name),
            isa_opcode=isa_opcode,
            ins=ins,
            outs=outs,
            perf_max=1,
        )
    )


# ------------------------------------------------------------- device build --

def _build_nc(reps=1):
    key = ("nc", reps)
    if key in _CACHE:
        return _CACHE[key]
    import concourse.bacc as bacc
    import concourse.mybir as mybir
    from concourse.tile import TileContext

    FUSED = _get_fused_op()
    f32 = mybir.dt.float32
    f16 = mybir.dt.float16
    bf16 = mybir.dt.bfloat16

    RAW_SLOTS = BPC * NT * NH            # row-min slots
    CM_SLOTS = BPC * NH                  # column-min output tiles

    nc = bacc.Bacc(None)
    ltab = nc.dram_tensor("ltab", [BPC, K, N], bf16, kind="ExternalInput")
    rtab = nc.dram_tensor("rtab", [BPC, K, M], bf16, kind="ExternalInput")
    # 2x-mode accum writeback is 16-bit; direct 1x tiles use fp32 slots.
    outr = nc.dram_tensor("raw", [128, RAW_SLOTS], f32, kind="ExternalOutput")
    outr16 = nc.dram_tensor("raw16", [128, RAW_SLOTS], f16,
                            kind="ExternalOutput")
    outc = nc.dram_tensor("colm", [128, CM_SLOTS * HW_TILE], f16,
                          kind="ExternalOutput")
    outc2 = nc.dram_tensor("colm2", [128, BPC * M], f16,
                           kind="ExternalOutput")

    with TileContext(nc) as tc:
        with (
            tc.tile_pool(name="stage", bufs=2) as stage,
            tc.tile_pool(name="psum", bufs=2, space="PSUM") as psum,
            tc.tile_pool(name="fp16", bufs=4) as fpool,
            tc.tile_pool(name="res", bufs=1) as res,
        ):
            raw = res.tile([128, RAW_SLOTS], f32, tag="raw")
            raw16 = res.tile([128, RAW_SLOTS], f16, tag="raw16")
            nc.vector.memset(raw[:, :], 3.0e38)
            nc.vector.memset(raw16[:, :], 60000.0)
            # direct-path chains: per (batch, m-half), [128, 2048] ping-pong
            cms = [res.tile([128, HW_TILE], f16, tag=f"cm{s}_{pp}",
                            name=f"cm{s}_{pp}")
                   for s in range(CM_SLOTS) for pp in range(2)]
            # offload-path chains: per batch, [128, 4096] ping-pong
            cws = [res.tile([128, M], f16, tag=f"cw{lb}_{pp}",
                            name=f"cw{lb}_{pp}")
                   for lb in range(BPC) for pp in range(2)]
            for t in cms + cws:
                nc.vector.memset(t[:, :], 60000.0)

            noff = [0, 0]                  # offloaded-op count per batch
            ndir = [[0, 0], [0, 0]]        # direct-op count per (batch, half)
            for _rep in range(reps):
              for lb in range(BPC):
                lt = stage.tile([K, N], bf16, tag="lt")
                rt = stage.tile([K, M], bf16, tag="rt")
                nc.sync.dma_start(out=lt[:, :], in_=ltab[lb])
                nc.sync.dma_start(out=rt[:, :], in_=rtab[lb])
                for i in range(NT):
                    ltT = lt[:, i * 128:(i + 1) * 128]
                    off = _is_offload(i)
                    fb = (fpool.tile([128, M], f16, tag="fb", name="fb")
                          if off else None)
                    for h in range(NH):
                        pa = psum.tile([128, HW_TILE], f32, tag="pa")
                        base = h * HW_TILE
                        for j in range(HW_TILE // 512):
                            nc.tensor.matmul(
                                pa[:, j * 512:(j + 1) * 512], ltT,
                                rt[:, base + j * 512:base + (j + 1) * 512],
                                start=True, stop=True)
                        slot = (lb * NT + i) * NH + h
                        if off:
                            nc.scalar.copy(
                                out=fb[:, base:base + HW_TILE], in_=pa[:, :])
                            if h == NH - 1:
                                pp = noff[lb] % 2
                                noff[lb] += 1
                                _emit_fused(
                                    nc, FUSED,
                                    out=cws[lb * 2 + 1 - pp][:, :],
                                    in0=fb[:, :],
                                    in1=cws[lb * 2 + pp][:, :],
                                    s0=3.0e38,
                                    accum_out=raw16[:, slot:slot + 1],
                                )
                        else:
                            cslot = (lb * NH + h) * 2
                            pp = ndir[lb][h] % 2
                            ndir[lb][h] += 1
                            _emit_fused(
                                nc, FUSED,
                                out=cms[cslot + 1 - pp][:, :],
                                in0=pa[:, :],
                                in1=cms[cslot + pp][:, :],
                                s0=3.0e38,
                                accum_out=raw[:, slot:slot + 1],
                            )
            # final column-min partials: parity = op-count % 2
            for s in range(CM_SLOTS):
                lb, h = divmod(s, NH)
                fin = cms[s * 2 + (ndir[lb][h] % 2)]
                nc.sync.dma_start(
                    out=outc[:, s * HW_TILE:(s + 1) * HW_TILE], in_=fin[:, :])
            for lb in range(BPC):
                fin = cws[lb * 2 + (noff[lb] % 2)]
                nc.sync.dma_start(
                    out=outc2[:, lb * M:(lb + 1) * M], in_=fin[:, :])
            nc.sync.dma_start(out=outr[:, :], in_=raw[:, :])
            nc.sync.dma_start(out=outr16[:, :], in_=raw16[:, :])
    nc.compile()
    _CACHE[key] = nc
    return nc


# -------------------------------------------------------------------- entry --

def _prepare_inputs(pred, target):
    ltabs = np.empty((NCORES, BPC, K, N), BF)
    rtabs = np.empty((NCORES, BPC, K, M), BF)
    for core in range(NCORES):
        for lb in range(BPC):
            b = core * BPC + lb
            lt, rt = _build_tabs(pred[b], target[b])
            ltabs[core, lb] = lt
            rtabs[core, lb] = rt
    return ltabs, rtabs


def _postprocess(results):
    losses = []
    for core in range(NCORES):
        raw = np.minimum(
            np.asarray(results[core]["raw"], np.float64),
            np.asarray(results[core]["raw16"], np.float64))   # (128, RAW_SLOTS)
        colm = np.asarray(results[core]["colm"], np.float64)  # (128, CM*2048)
        colm2 = np.asarray(results[core]["colm2"], np.float64)  # (128, BPC*M)
        for lb in range(BPC):
            sl = raw[:, lb * NT * NH:(lb + 1) * NT * NH]
            rowmin = sl.reshape(128, NT, NH).min(axis=2)      # (p, i)
            rowmin = rowmin.T.reshape(-1)                     # n = i*128 + p
            cm = colm[:, lb * NH * HW_TILE:(lb + 1) * NH * HW_TILE]
            cm2 = colm2[:, lb * M:(lb + 1) * M]
            colmin = np.minimum(cm.min(axis=0), cm2.min(axis=0))  # (M,)
            losses.append(rowmin.mean() + colmin.mean())
    return np.float32(np.mean(losses))


def _run(pred, target, trace=False):
    from concourse.bass_utils import run_bass_kernel_spmd

    pred = np.asarray(pred, dtype=np.float32)
    target = np.asarray(target, dtype=np.float32)
    assert pred.shape == (B, N, C) and target.shape == (B, M, C)
    ltabs, rtabs = _prepare_inputs(pred, target)
    nc = _build_nc()
    in_maps = [{"ltab": ltabs[c], "rtab": rtabs[c]} for c in range(NCORES)]
    res = run_bass_kernel_spmd(nc, in_maps, core_ids=list(range(NCORES)), trace=trace)
    return _postprocess(res.results), res


def kernel(pred, target):
    loss, _ = _run(pred, target, trace=False)
    return loss
